# revision 1
# baseline (speedup 1.0000x reference)
"""Trainium2 Bass kernel for nn_DecomposedKLDAddLoss.

Reference computes, for z, loc, scale in [B, D]:
    mi  = mean(log_qz_cond_x - log_qz)
    tc  = mean(log_qz - log_qz_prod)
    kl  = mean(log_qz_prod - log_pz)
    out = 1.0*mi + 1.0*tc + 1.0*kl
With unit weights the sum telescopes exactly: log_qz and log_qz_prod
(the only terms needing the [B,B,D] pairwise matrix) cancel, leaving
    out = mean_i(log_qz_cond_x[i] - log_pz[i])
        = (1/B) * sum_{i,d} [ 0.5*z^2 - 0.5*((z-loc)/scale)^2 - ln(scale) ]
(the -0.5*log(2*pi) terms also cancel elementwise).  Measured against
the fp32 reference this matches to ~1e-7 relative, the same error an
exact f64 evaluation of the full decomposition has, because the
reference's own rounding in log_qz / log_qz_prod cancels between terms.

Sharding: rows of z/loc/scale are split evenly across the 8 cores (256
rows each).  The host packs each core's shard into one [128, 387] f32
block, contiguous per partition:
    [ +1/B | -1/B | 0.0 | scale (2 row-blocks) | z (2) | loc (2) ]
so the load is a single large-descriptor DMA.  Each core reduces its
shard to a scalar partial (sum/B over its rows) written to its own
output; the partials are summed while unsharding (the output is
sum-sharded across cores).

Raw Bass (no Tile): the per-partition row sums go through a pair of
accumulating 128x1 matmuls with +1/B / -1/B weight columns, PSUM ->
SBUF via the scalar engine, one 4-byte DMA out.  A dummy Ln activation
before the input-DMA wait pulls the ~1.3us ACT table load off the
critical path.
"""

import numpy as np

import concourse.bass as bass
import concourse.mybir as mybir
from concourse.bass_utils import run_bass_kernel_spmd

N_CORES = 8
B, D = 2048, 64
SH = B // N_CORES   # 256 rows per core
P = 128             # SBUF partition count
NB = SH // P        # 2 row-blocks of 128 rows per tensor per core
F = NB * D          # 128 free elements per partition per tensor
NCONST = 3          # +1/B | -1/B | 0.0
W = 3 * F + NCONST
F32 = mybir.dt.float32

_CACHE: dict = {}


def _build_nc():
    nc = bass.Bass(
        "TRN2",
        target_bir_lowering=False,
        debug=False,
        enable_asserts=False,
        num_devices=N_CORES,
    )
    in_ext = nc.dram_tensor("zls", [P, W], F32, kind="ExternalInput").ap()
    out_ext = nc.dram_tensor("out", [1, 1], F32, kind="ExternalOutput").ap()

    mult = mybir.AluOpType.mult

    from contextlib import ExitStack

    with ExitStack() as ctx:
        big = ctx.enter_context(nc.sbuf_tensor([P, W], F32))
        rs = ctx.enter_context(nc.sbuf_tensor([P, F], F32))
        df = ctx.enter_context(nc.sbuf_tensor([P, F], F32))
        tt = ctx.enter_context(nc.sbuf_tensor([P, F], F32))
        z2 = ctx.enter_context(nc.sbuf_tensor([P, F], F32))
        t2 = ctx.enter_context(nc.sbuf_tensor([P, F], F32))
        lnt = ctx.enter_context(nc.sbuf_tensor([P, F], F32))
        acc_z = ctx.enter_context(nc.sbuf_tensor([P, 1], F32))
        acc_t = ctx.enter_context(nc.sbuf_tensor([P, 1], F32))
        acc_zt = ctx.enter_context(nc.sbuf_tensor([P, 1], F32))
        ln_acc = ctx.enter_context(nc.sbuf_tensor([P, 1], F32))
        dum = ctx.enter_context(nc.sbuf_tensor([1, 2], F32))
        dumo = ctx.enter_context(nc.sbuf_tensor([1, 1], F32))
        stage = ctx.enter_context(nc.sbuf_tensor([1, 1], F32))
        pt = ctx.enter_context(nc.psum_tensor([1, 1], F32))
        s_d1 = ctx.enter_context(nc.semaphore("s_d1"))
        s_v = ctx.enter_context(nc.semaphore("s_v"))
        s_a = ctx.enter_context(nc.semaphore("s_a"))
        s_mm = ctx.enter_context(nc.semaphore("s_mm"))
        s_st = ctx.enter_context(nc.semaphore("s_st"))
        block = ctx.enter_context(nc.Block())
        wgt_p = big[:, 0:1]    # +1/B
        wgt_n = big[:, 1:2]    # -1/B
        zbias = big[:, 2:3]    # 0.0 (Ln activation bias)
        st = big[:, NCONST : NCONST + F]
        zt = big[:, NCONST + F : NCONST + 2 * F]
        lt = big[:, NCONST + 2 * F : NCONST + 3 * F]

        @block.sync
        def _(sync):
            sync.dma_start(out=big[:], in_=in_ext).then_inc(s_d1, 16)
            sync.wait_ge(s_st, 1)
            sync.dma_start(out=out_ext, in_=stage[:], single_packet=True).then_inc(
                s_d1, 16
            )

        @block.vector
        def _(v):
            v.wait_ge(s_d1, 16)
            v.reciprocal(rs[:], st).then_inc(s_v, 1)            # 1
            v.tensor_sub(df[:], zt, lt).then_inc(s_v, 1)        # 2
            v.scalar_tensor_tensor(
                z2[:], zt, 0.5, zt, op0=mult, op1=mult, accum_out=acc_z[:]
            ).then_inc(s_v, 1)                                  # 3
            v.wait_ge(s_v, 2)
            v.tensor_mul(tt[:], df[:], rs[:]).then_inc(s_v, 1)  # 4
            v.wait_ge(s_v, 4)
            v.scalar_tensor_tensor(
                t2[:], tt[:], -0.5, tt[:], op0=mult, op1=mult, accum_out=acc_t[:]
            ).then_inc(s_v, 1)                                  # 5
            v.wait_ge(s_v, 5)
            v.tensor_add(acc_zt[:], acc_z[:], acc_t[:]).then_inc(s_v, 1)  # 6

        @block.gpsimd
        def _(g):
            g.memset(dum[:], 1.0).then_inc(s_a, 1)

        @block.scalar
        def _(a):
            # dummy Ln loads the ACT function table before the DMA wait
            a.wait_ge(s_a, 1)
            a.activation(dumo[:], dum[:, 0:1], mybir.ActivationFunctionType.Ln,
                         bias=dum[:, 1:2])
            a.wait_ge(s_d1, 16)
            a.activation(
                lnt[:],
                st,
                mybir.ActivationFunctionType.Ln,
                bias=zbias,
                accum_out=ln_acc[:],
            ).then_inc(s_a, 1)  # s_a == 2
            a.wait_ge(s_mm, 1)
            a.copy(stage[:], pt[:]).then_inc(s_st, 1)

        @block.tensor
        def _(t):
            t.wait_ge(s_v, 6)
            t.wait_ge(s_a, 2)
            # pt = sum_p(acc_zt)/B - sum_p(ln_acc)/B
            t.matmul(pt[:], lhsT=wgt_p, rhs=acc_zt[:], start=True, stop=False)
            t.matmul(pt[:], lhsT=wgt_n, rhs=ln_acc[:], start=False, stop=True).then_inc(
                s_mm, 1
            )

    return nc


def _get_nc():
    if "nc" not in _CACHE:
        _CACHE["nc"] = _build_nc()
    return _CACHE["nc"]


def _in_maps(z, loc, scale):
    z = np.asarray(z, dtype=np.float32)
    loc = np.asarray(loc, dtype=np.float32)
    scale = np.asarray(scale, dtype=np.float32)
    consts = np.zeros((P, NCONST), dtype=np.float32)
    consts[:, 0] = 1.0 / B
    consts[:, 1] = -1.0 / B
    maps = []
    for c in range(N_CORES):
        blocks = [consts]
        for t in (scale, z, loc):
            sh = t[c * SH : (c + 1) * SH]
            blocks.extend(sh[n * P : (n + 1) * P] for n in range(NB))
        maps.append({"zls": np.hstack(blocks)})
    return maps


def _combine(results):
    # output is sum-sharded: unshard by summing the 8 partial scalars
    return np.array(
        np.sum([results[c]["out"][0, 0] for c in range(N_CORES)], dtype=np.float32),
        dtype=np.float32,
    )


def run_traced(z, loc, scale, tmpdir=None):
    """Run with NTFF profiling; returns (value, BassKernelResults)."""
    res = run_bass_kernel_spmd(
        _get_nc(), _in_maps(z, loc, scale), list(range(N_CORES)),
        trace=True, tmpdir=tmpdir,
    )
    return _combine(res.results), res


def kernel(z, loc, scale):
    res = run_bass_kernel_spmd(
        _get_nc(), _in_maps(z, loc, scale), list(range(N_CORES))
    )
    return _combine(res.results)



# revision 2
# speedup vs baseline: 1.3644x; 1.3644x over previous
"""Trainium2 Bass kernel for nn_DecomposedKLDAddLoss.

Reference computes, for z, loc, scale in [B, D]:
    mi  = mean(log_qz_cond_x - log_qz)
    tc  = mean(log_qz - log_qz_prod)
    kl  = mean(log_qz_prod - log_pz)
    out = 1.0*mi + 1.0*tc + 1.0*kl
With unit weights the sum telescopes exactly: log_qz and log_qz_prod
(the only terms needing the [B,B,D] pairwise matrix) cancel, leaving
    out = mean_i(log_qz_cond_x[i] - log_pz[i])
        = (1/B) * sum_{i,d} [ 0.5*z^2 - 0.5*((z-loc)/scale)^2 - ln(scale) ]
(the -0.5*log(2*pi) terms also cancel elementwise).

Sharding: rows are split across the 8 cores (256 rows each, folded to
[128 partitions x 2 row-blocks]).  Inputs are packed host-side as fp16
(tolerance is 2e-2; measured error vs the fp32 reference is ~3e-5),
halving HBM traffic.

Per-core dataflow (engines run concurrently):
  sync   : DMA scale chunk in; DMA the [128,3] accumulator strip out.
  scalar : DMA z|loc chunk in (second HWDGE queue, parallel with sync's);
           dummy activation pulls the ~1.3us ln/exp ACT table load off the
           critical path; ln(s) with row-accumulate; q = exp(-2 ln s)
           = 1/s^2 -- the scalar engine replaces the slow (~950ns) DVE
           reciprocal entirely.
  vector : d = z - loc; d2h = -0.5*d^2; 0.5*z^2 row-accumulated;
           t2 = d2h*q row-accumulated; a [128,1] fence copy that reads the
           last accumulator column carries the completion semaphore, so
           the out-DMA can never observe a stale accumulator.
The three row-sum columns (0.5z^2 | ln s | -0.5((z-loc)/s)^2) are
combined on the host while unsharding (output is sum-sharded).
"""

import numpy as np

import concourse.bass as bass
import concourse.mybir as mybir
from concourse.bass_utils import run_bass_kernel_spmd

N_CORES = 8
B, D = 2048, 64
SH = B // N_CORES   # 256 rows per core
P = 128             # SBUF partition count
NB = SH // P        # 2 row-blocks of 128 rows per tensor per core
F = NB * D          # 128 free elements per partition per tensor
F32 = mybir.dt.float32
FP16 = mybir.dt.float16

_CACHE: dict = {}


def _build_nc():
    mult = mybir.AluOpType.mult
    Ln = mybir.ActivationFunctionType.Ln
    Exp = mybir.ActivationFunctionType.Exp

    nc = bass.Bass(
        "TRN2",
        target_bir_lowering=False,
        debug=False,
        enable_asserts=False,
        num_devices=N_CORES,
    )
    s_ext = nc.dram_tensor("s", [P, F], FP16, kind="ExternalInput").ap()
    zl_ext = nc.dram_tensor("zl", [P, 2 * F], FP16, kind="ExternalInput").ap()
    out_ext = nc.dram_tensor("out", [P, 3], F32, kind="ExternalOutput").ap()

    from contextlib import ExitStack

    with ExitStack() as ctx:
        sb_s = ctx.enter_context(nc.sbuf_tensor([P, F], FP16))
        sb_zl = ctx.enter_context(nc.sbuf_tensor([P, 2 * F], FP16))
        lns = ctx.enter_context(nc.sbuf_tensor([P, F], F32))
        q = ctx.enter_context(nc.sbuf_tensor([P, F], FP16))
        dd = ctx.enter_context(nc.sbuf_tensor([P, F], FP16))
        d2h = ctx.enter_context(nc.sbuf_tensor([P, F], FP16))
        zz = ctx.enter_context(nc.sbuf_tensor([P, F], FP16))
        tt = ctx.enter_context(nc.sbuf_tensor([P, F], FP16))
        accs = ctx.enter_context(nc.sbuf_tensor([P, 4], F32))
        dumo = ctx.enter_context(nc.sbuf_tensor([1, 1], F32))
        fence = ctx.enter_context(nc.sbuf_tensor([P, 1], F32))
        s_s = ctx.enter_context(nc.semaphore("s_s"))
        s_zl = ctx.enter_context(nc.semaphore("s_zl"))
        s_q = ctx.enter_context(nc.semaphore("s_q"))
        s_t2 = ctx.enter_context(nc.semaphore("s_t2"))
        s_o = ctx.enter_context(nc.semaphore("s_o"))
        block = ctx.enter_context(nc.Block(no_gpsimd_drain=True))
        zt = sb_zl[:, 0:F]
        lt = sb_zl[:, F:2 * F]
        one_col = nc.const_aps.aps[(F32, 1.0)]
        # accs columns: 0 = sum 0.5*z^2, 1 = sum ln s, 2 = sum -0.5*t^2

        @block.sync
        def _(sync):
            sync.dma_start(out=sb_s[:], in_=s_ext).then_inc(s_s, 16)
            sync.wait_ge(s_t2, 1)
            sync.wait_ge(s_q, 1)
            sync.dma_start(out=out_ext, in_=accs[:, 0:3]).then_inc(s_o, 16)

        @block.scalar
        def _(a):
            a.dma_start(out=sb_zl[:], in_=zl_ext).then_inc(s_zl, 16)
            a.activation(dumo[:], one_col[0:1, 0:1], Exp)
            a.wait_ge(s_s, 16)
            a.activation(lns[:], sb_s[:], Ln, accum_out=accs[:, 1:2])
            # s_q also orders the ln accumulator read (same engine, earlier
            # in the stream) ahead of the out-DMA
            a.activation(q[:], lns[:], Exp, scale=-2.0).then_inc(s_q, 1)

        @block.vector
        def _(v):
            v.wait_ge(s_zl, 16)
            v.tensor_sub(dd[:], zt, lt)
            v.scalar_tensor_tensor(d2h[:], dd[:], -0.5, dd[:], op0=mult,
                                   op1=mult)
            v.scalar_tensor_tensor(zz[:], zt, 0.5, zt, op0=mult, op1=mult,
                                   accum_out=accs[:, 0:1])
            v.wait_ge(s_q, 1)
            v.scalar_tensor_tensor(tt[:], d2h[:], 1.0, q[:], op0=mult,
                                   op1=mult, accum_out=accs[:, 2:3])
            v.tensor_scalar_mul(fence[:], accs[:, 2:3], 1.0).then_inc(s_t2, 1)

    return nc


def _get_nc():
    if "nc" not in _CACHE:
        _CACHE["nc"] = _build_nc()
    return _CACHE["nc"]


def _in_maps(z, loc, scale):
    z = np.asarray(z, dtype=np.float32)
    loc = np.asarray(loc, dtype=np.float32)
    scale = np.asarray(scale, dtype=np.float32)
    maps = []
    for c in range(N_CORES):
        sh_s = scale[c * SH:(c + 1) * SH]
        sh_z = z[c * SH:(c + 1) * SH]
        sh_l = loc[c * SH:(c + 1) * SH]
        s_blk = np.hstack([sh_s[n * P:(n + 1) * P] for n in range(NB)])
        z_blk = np.hstack([sh_z[n * P:(n + 1) * P] for n in range(NB)])
        l_blk = np.hstack([sh_l[n * P:(n + 1) * P] for n in range(NB)])
        maps.append({
            "s": s_blk.astype(np.float16),
            "zl": np.hstack([z_blk, l_blk]).astype(np.float16),
        })
    return maps


def _combine(results):
    # output is sum-sharded: unshard by summing the per-partition row sums
    # of all 8 cores; signs/scaling folded here
    tot = np.float64(0.0)
    for c in range(N_CORES):
        o = results[c]["out"].astype(np.float64)
        tot += np.sum(o[:, 0]) - np.sum(o[:, 1]) + np.sum(o[:, 2])
    return np.array(tot / B, dtype=np.float32)


def run_traced(z, loc, scale, tmpdir=None):
    """Run with NTFF profiling; returns (value, BassKernelResults)."""
    res = run_bass_kernel_spmd(
        _get_nc(), _in_maps(z, loc, scale), list(range(N_CORES)),
        trace=True, tmpdir=tmpdir,
    )
    return _combine(res.results), res


def kernel(z, loc, scale):
    res = run_bass_kernel_spmd(
        _get_nc(), _in_maps(z, loc, scale), list(range(N_CORES))
    )
    return _combine(res.results)


# revision 3
# speedup vs baseline: 1.3729x; 1.0063x over previous
"""Trainium2 Bass kernel for nn_DecomposedKLDAddLoss.

Reference computes, for z, loc, scale in [B, D]:
    mi  = mean(log_qz_cond_x - log_qz)
    tc  = mean(log_qz - log_qz_prod)
    kl  = mean(log_qz_prod - log_pz)
    out = 1.0*mi + 1.0*tc + 1.0*kl
With unit weights the sum telescopes exactly: log_qz and log_qz_prod
(the only terms needing the [B,B,D] pairwise matrix) cancel, leaving
    out = mean_i(log_qz_cond_x[i] - log_pz[i])
        = (1/B) * sum_{i,d} [ 0.5*z^2 - 0.5*((z-loc)/scale)^2 - ln(scale) ]
(the -0.5*log(2*pi) terms also cancel elementwise).

Sharding: rows are split across the 8 cores (256 rows each, folded to
[128 partitions x 2 row-blocks]).  Inputs are packed host-side as fp16
(tolerance is 2e-2; measured error vs the fp32 reference is ~3e-5),
halving HBM traffic.

Per-core dataflow (engines run concurrently):
  sync   : DMA scale chunk in; DMA the [128,3] accumulator strip out.
  scalar : DMA z|loc chunk in (second HWDGE queue, parallel with sync's);
           dummy activation pulls the ~1.3us ln/exp ACT table load off the
           critical path; ln(s) with row-accumulate; q = exp(-2 ln s)
           = 1/s^2 -- the scalar engine replaces the slow (~950ns) DVE
           reciprocal entirely.
  vector : d = z - loc; d2h = -0.5*d^2; 0.5*z^2 row-accumulated;
           t2 = d2h*q row-accumulated; a [128,1] fence copy that reads the
           last accumulator column carries the completion semaphore, so
           the out-DMA can never observe a stale accumulator.
The three row-sum columns (0.5z^2 | ln s | -0.5((z-loc)/s)^2) are
combined on the host while unsharding (output is sum-sharded).
"""

import numpy as np

import concourse.bass as bass
import concourse.mybir as mybir
from concourse.bass_utils import run_bass_kernel_spmd

N_CORES = 8
B, D = 2048, 64
SH = B // N_CORES   # 256 rows per core
P = 128             # SBUF partition count
NB = SH // P        # 2 row-blocks of 128 rows per tensor per core
F = NB * D          # 128 free elements per partition per tensor
F32 = mybir.dt.float32
FP16 = mybir.dt.float16

_CACHE: dict = {}


def _build_nc():
    mult = mybir.AluOpType.mult
    Ln = mybir.ActivationFunctionType.Ln
    Exp = mybir.ActivationFunctionType.Exp

    nc = bass.Bass(
        "TRN2",
        target_bir_lowering=False,
        debug=False,
        enable_asserts=False,
        num_devices=N_CORES,
    )
    s_ext = nc.dram_tensor("s", [P, F], FP16, kind="ExternalInput").ap()
    zl_ext = nc.dram_tensor("zl", [P, 2 * F], FP16, kind="ExternalInput").ap()
    out_ext = nc.dram_tensor("out", [P, 3], F32, kind="ExternalOutput").ap()

    from contextlib import ExitStack

    with ExitStack() as ctx:
        sb_s = ctx.enter_context(nc.sbuf_tensor([P, F], FP16))
        sb_zl = ctx.enter_context(nc.sbuf_tensor([P, 2 * F], FP16))
        lns = ctx.enter_context(nc.sbuf_tensor([P, F], F32))
        q = ctx.enter_context(nc.sbuf_tensor([P, F], FP16))
        dd = ctx.enter_context(nc.sbuf_tensor([P, F], FP16))
        d2h = ctx.enter_context(nc.sbuf_tensor([P, F], FP16))
        zz = ctx.enter_context(nc.sbuf_tensor([P, F], FP16))
        tt = ctx.enter_context(nc.sbuf_tensor([P, F], FP16))
        accs = ctx.enter_context(nc.sbuf_tensor([P, 4], F32))
        dumo = ctx.enter_context(nc.sbuf_tensor([1, 1], F32))
        fence = ctx.enter_context(nc.sbuf_tensor([P, 1], F32))
        s_s = ctx.enter_context(nc.semaphore("s_s"))
        s_zl = ctx.enter_context(nc.semaphore("s_zl"))
        s_q = ctx.enter_context(nc.semaphore("s_q"))
        s_t2 = ctx.enter_context(nc.semaphore("s_t2"))
        s_o = ctx.enter_context(nc.semaphore("s_o"))
        block = ctx.enter_context(nc.Block(no_gpsimd_drain=True))
        zt = sb_zl[:, 0:F]
        lt = sb_zl[:, F:2 * F]
        one_col = nc.const_aps.aps[(F32, 1.0)]
        # accs columns: 0 = sum 0.5*z^2, 1 = sum ln s, 2 = sum -0.5*t^2

        @block.sync
        def _(sync):
            sync.dma_start(out=sb_s[:], in_=s_ext).then_inc(s_s, 16)
            # s_t2 (vector fence) transitively orders the scalar chain too:
            # the t2 stt waits s_q, which the Exp increments after the ln
            # accumulator read has retired on the scalar stream
            sync.wait_ge(s_t2, 1)
            sync.dma_start(out=out_ext, in_=accs[:, 0:3]).then_inc(s_o, 16)

        @block.scalar
        def _(a):
            a.dma_start(out=sb_zl[:], in_=zl_ext).then_inc(s_zl, 16)
            a.activation(dumo[:], one_col[0:1, 0:1], Exp)
            a.wait_ge(s_s, 16)
            a.activation(lns[:], sb_s[:], Ln, accum_out=accs[:, 1:2])
            # s_q also orders the ln accumulator read (same engine, earlier
            # in the stream) ahead of the out-DMA
            a.activation(q[:], lns[:], Exp, scale=-2.0).then_inc(s_q, 1)

        @block.vector
        def _(v):
            v.wait_ge(s_zl, 16)
            v.tensor_sub(dd[:], zt, lt)
            v.scalar_tensor_tensor(d2h[:], dd[:], -0.5, dd[:], op0=mult,
                                   op1=mult)
            v.scalar_tensor_tensor(zz[:], zt, 0.5, zt, op0=mult, op1=mult,
                                   accum_out=accs[:, 0:1])
            v.wait_ge(s_q, 1)
            v.scalar_tensor_tensor(tt[:], d2h[:], 1.0, q[:], op0=mult,
                                   op1=mult, accum_out=accs[:, 2:3])
            v.tensor_scalar_mul(fence[:], accs[:, 2:3], 1.0).then_inc(s_t2, 1)

    return nc


def _get_nc():
    if "nc" not in _CACHE:
        _CACHE["nc"] = _build_nc()
    return _CACHE["nc"]


def _in_maps(z, loc, scale):
    z = np.asarray(z, dtype=np.float32)
    loc = np.asarray(loc, dtype=np.float32)
    scale = np.asarray(scale, dtype=np.float32)
    maps = []
    for c in range(N_CORES):
        sh_s = scale[c * SH:(c + 1) * SH]
        sh_z = z[c * SH:(c + 1) * SH]
        sh_l = loc[c * SH:(c + 1) * SH]
        s_blk = np.hstack([sh_s[n * P:(n + 1) * P] for n in range(NB)])
        z_blk = np.hstack([sh_z[n * P:(n + 1) * P] for n in range(NB)])
        l_blk = np.hstack([sh_l[n * P:(n + 1) * P] for n in range(NB)])
        maps.append({
            "s": s_blk.astype(np.float16),
            "zl": np.hstack([z_blk, l_blk]).astype(np.float16),
        })
    return maps


def _combine(results):
    # output is sum-sharded: unshard by summing the per-partition row sums
    # of all 8 cores; signs/scaling folded here
    tot = np.float64(0.0)
    for c in range(N_CORES):
        o = results[c]["out"].astype(np.float64)
        tot += np.sum(o[:, 0]) - np.sum(o[:, 1]) + np.sum(o[:, 2])
    return np.array(tot / B, dtype=np.float32)


def run_traced(z, loc, scale, tmpdir=None):
    """Run with NTFF profiling; returns (value, BassKernelResults)."""
    res = run_bass_kernel_spmd(
        _get_nc(), _in_maps(z, loc, scale), list(range(N_CORES)),
        trace=True, tmpdir=tmpdir,
    )
    return _combine(res.results), res


def kernel(z, loc, scale):
    res = run_bass_kernel_spmd(
        _get_nc(), _in_maps(z, loc, scale), list(range(N_CORES))
    )
    return _combine(res.results)
